# revision 5
# baseline (speedup 1.0000x reference)
"""KV-cache scatter-update kernel for Trainium2, SPMD across 8 NeuronCores.

Problem nn_KVCache_16939351015933:
  out = concat(cache[:, :1024], cache[:, 1024:1152] + x)   (seq axis)
with static index=1024, reset_index=0, L=128. The masks do not affect the
returned content. Sharding: batch (B=8) across 8 cores, fully local.

Per-core work: copy 16.78 MB (rows 0:1024) DRAM->DRAM, plus a 2 MB add
(rows 1024:1152) staged through SBUF with the DMA inline-ALU (accum_op=add).
Pure-DMA kernel, HBM-bound: ~40 MB traffic/core -> ~111 us roofline.
"""

import sys

import numpy as np

sys.path.insert(0, "/opt/trn_rl_repo")

import concourse.bass as bass
import concourse.mybir as mybir
from concourse.bass_utils import run_bass_kernel_spmd

B, S, H, D = 8, 4096, 32, 128
L = 128          # new chunk length
IDX = 1024       # static cache write offset
TO = IDX + L     # output seq length (1152)
F = H * D        # 4096 floats per (batch, seq) position = 16 KB
N_CORES = 8

N_COPY_CHUNKS = 4   # DRAM->DRAM copy split (rows 0:1024)

_NC = None


def _build() -> bass.Bass:
    nc = bass.Bass()
    cache = nc.dram_tensor("cache", [TO, F], mybir.dt.float32, kind="ExternalInput")
    x = nc.dram_tensor("x", [L, F], mybir.dt.float32, kind="ExternalInput")
    out = nc.dram_tensor("out", [TO, F], mybir.dt.float32, kind="ExternalOutput")

    with (
        nc.sbuf_tensor([L, F], mybir.dt.float32) as a,
        nc.sbuf_tensor([L, F], mybir.dt.float32) as b,
        nc.sbuf_tensor([L, F], mybir.dt.float32) as c,
        nc.semaphore() as s_load,
        nc.semaphore() as s_add,
        nc.semaphore() as s_all,
        nc.Block() as block,
    ):

        @block.sync
        def _(sp):
            # stage the to-be-updated rows and the update into SBUF
            sp.dma_start(out=a[:], in_=cache[IDX:TO, :]).then_inc(s_load, 16)
            sp.dma_start(out=b[:], in_=x[:, :]).then_inc(s_load, 16)
            # big DRAM->DRAM copy of the untouched prefix
            rows = IDX // N_COPY_CHUNKS
            for i in range(N_COPY_CHUNKS):
                rs = bass.ts(i, rows)
                sp.dma_start(out=out[rs, :], in_=cache[rs, :]).then_inc(s_all, 16)
            # store the updated rows once the add landed
            sp.wait_ge(s_add, 1)
            sp.dma_start(out=out[IDX:TO, :], in_=c[:]).then_inc(s_all, 16)
            sp.wait_ge(s_all, 16 * (N_COPY_CHUNKS + 1))

        @block.vector
        def _(v):
            # standalone waits: the TensorTensor itself carries no sem wait
            # (walrus caps non-EVSEM instructions at 1 wait slot)
            v.wait_ge(s_load, 32)
            v.tensor_add(c[:], a[:], b[:]).then_inc(s_add, 1)

    return nc


def kernel(cache, cache_mask, x, mask, index, reset_index, **_unused):
    global _NC
    assert int(index) == IDX and int(reset_index) == 0
    cache = np.asarray(cache, dtype=np.float32)
    x = np.asarray(x, dtype=np.float32)
    # Batch-shard: core i owns batch i. Only rows < TO are ever read.
    cache_s = np.ascontiguousarray(cache[:, :TO]).reshape(B, TO, F)
    x_s = np.ascontiguousarray(x).reshape(B, L, F)
    if _NC is None:
        _NC = _build()
    in_maps = [{"cache": cache_s[i], "x": x_s[i]} for i in range(N_CORES)]
    res = run_bass_kernel_spmd(_NC, in_maps, core_ids=list(range(N_CORES)))
    out = np.stack([res.results[i]["out"] for i in range(N_CORES)])
    return out.reshape(B, TO, H, D)


# revision 6
# speedup vs baseline: 9.7840x; 9.7840x over previous
"""KV-cache scatter-update kernel for Trainium2, SPMD across 8 NeuronCores.

Problem nn_KVCache_16939351015933:
  out = concat(cache[:, :1024], cache[:, 1024:1152] + x)   (seq axis)
with static index=1024, reset_index=0, L=128. The masks do not affect the
returned content. Sharding: batch (B=8) across 8 cores, fully local.

Per-core work: copy 16.78 MB (rows 0:1024) DRAM->DRAM, plus a 2 MB add
(rows 1024:1152) staged through SBUF with the DMA inline-ALU (accum_op=add).
Pure-DMA kernel, HBM-bound: ~40 MB traffic/core -> ~111 us roofline.
"""

import sys

import numpy as np

sys.path.insert(0, "/opt/trn_rl_repo")

import concourse.bass as bass
import concourse.mybir as mybir
from concourse.bass_utils import run_bass_kernel_spmd

B, S, H, D = 8, 4096, 32, 128
L = 128          # new chunk length
IDX = 1024       # static cache write offset
TO = IDX + L     # output seq length (1152)
F = H * D        # 4096 floats per (batch, seq) position = 16 KB
N_CORES = 8

N_COPY_CHUNKS = 4   # DRAM->DRAM copy split (rows 0:1024)

_NC = None


def _build(repeats: int = 1) -> bass.Bass:
    """repeats > 1 serializes the whole body R times — timing-only variant
    to separate device exec time from host dispatch overhead."""
    nc = bass.Bass()
    cache = nc.dram_tensor("cache", [TO, F], mybir.dt.float32, kind="ExternalInput")
    x = nc.dram_tensor("x", [L, F], mybir.dt.float32, kind="ExternalInput")
    out = nc.dram_tensor("out", [TO, F], mybir.dt.float32, kind="ExternalOutput")

    with (
        nc.sbuf_tensor([L, F], mybir.dt.float32) as a,
        nc.sbuf_tensor([L, F], mybir.dt.float32) as b,
        nc.sbuf_tensor([L, F], mybir.dt.float32) as c,
        nc.semaphore() as s_load,
        nc.semaphore() as s_add,
        nc.semaphore() as s_all,
        nc.Block() as block,
    ):

        @block.sync
        def _(sp):
            for r in range(repeats):
                if repeats > 1:
                    sp.wait_ge(s_all, 16 * (N_COPY_CHUNKS + 1) * r)
                # stage the to-be-updated rows and the update into SBUF
                sp.dma_start(out=a[:], in_=cache[IDX:TO, :]).then_inc(s_load, 16)
                sp.dma_start(out=b[:], in_=x[:, :]).then_inc(s_load, 16)
                # big DRAM->DRAM copy of the untouched prefix
                rows = IDX // N_COPY_CHUNKS
                for i in range(N_COPY_CHUNKS):
                    rs = bass.ts(i, rows)
                    sp.dma_start(out=out[rs, :], in_=cache[rs, :]).then_inc(
                        s_all, 16
                    )
                # store the updated rows once the add landed
                sp.wait_ge(s_add, r + 1)
                sp.dma_start(out=out[IDX:TO, :], in_=c[:]).then_inc(s_all, 16)
            sp.wait_ge(s_all, 16 * (N_COPY_CHUNKS + 1) * repeats)

        @block.vector
        def _(v):
            # standalone waits: the TensorTensor itself carries no sem wait
            # (walrus caps non-EVSEM instructions at 1 wait slot)
            for r in range(repeats):
                v.wait_ge(s_load, 32 * (r + 1))
                v.tensor_add(c[:], a[:], b[:]).then_inc(s_add, 1)

    return nc


def kernel(cache, cache_mask, x, mask, index, reset_index, **_unused):
    global _NC
    assert int(index) == IDX and int(reset_index) == 0
    cache = np.asarray(cache, dtype=np.float32)
    x = np.asarray(x, dtype=np.float32)
    # Batch-shard: core i owns batch i. Only rows < TO are ever read.
    cache_s = np.ascontiguousarray(cache[:, :TO]).reshape(B, TO, F)
    x_s = np.ascontiguousarray(x).reshape(B, L, F)
    if _NC is None:
        _NC = _build()
    in_maps = [{"cache": cache_s[i], "x": x_s[i]} for i in range(N_CORES)]
    res = run_bass_kernel_spmd(_NC, in_maps, core_ids=list(range(N_CORES)))
    out = np.stack([res.results[i]["out"] for i in range(N_CORES)])
    return out.reshape(B, TO, H, D)


# revision 7
# speedup vs baseline: 12.6038x; 1.2882x over previous
"""KV-cache scatter-update kernel for Trainium2, SPMD across 8 NeuronCores.

Problem nn_KVCache_16939351015933:
  out = concat(cache[:, :1024], cache[:, 1024:1152] + x)   (seq axis)
with static index=1024, reset_index=0, L=128. The masks do not affect the
returned content. Sharding: batch (B=8) across 8 cores, fully local.

Structure (per core, ~40 MB HBM traffic, sustained-shared-HBM bound):
  - SP ring:  ONE DRAM->DRAM copy of rows 0:1024 (16.78 MB). Consecutive
    DMAs on a ring serialize (~4-8 us each), so one chunk is fastest.
  - ACT ring: load cache[1024:1152] and x to SBUF, store the sum; overlaps
    the SP copy entirely (disjoint output rows).
  - DVE:      the add (TensorTensor carries no sem wait: walrus caps
    non-EVSEM instructions at 1 wait slot, so waits are standalone).
"""

import sys

import numpy as np

sys.path.insert(0, "/opt/trn_rl_repo")

import concourse.bass as bass
import concourse.mybir as mybir
from concourse.bass_utils import run_bass_kernel_spmd

B, S, H, D = 8, 4096, 32, 128
L = 128          # new chunk length
IDX = 1024       # static cache write offset
TO = IDX + L     # output seq length (1152)
F = H * D        # 4096 floats per (batch, seq) position = 16 KB
N_CORES = 8

_NC = None


def _build(repeats: int = 1) -> bass.Bass:
    """repeats > 1 serializes the whole body R times — timing-only variant
    to separate device exec time from host dispatch overhead."""
    nc = bass.Bass()
    cache = nc.dram_tensor("cache", [TO, F], mybir.dt.float32, kind="ExternalInput")
    x = nc.dram_tensor("x", [L, F], mybir.dt.float32, kind="ExternalInput")
    out = nc.dram_tensor("out", [TO, F], mybir.dt.float32, kind="ExternalOutput")

    with (
        nc.sbuf_tensor([L, F], mybir.dt.float32) as a,
        nc.sbuf_tensor([L, F], mybir.dt.float32) as b,
        nc.sbuf_tensor([L, F], mybir.dt.float32) as c,
        nc.semaphore() as s_load,
        nc.semaphore() as s_add,
        nc.semaphore() as s_all,
        nc.Block() as block,
    ):

        @block.sync
        def _(sp):
            # one big DRAM->DRAM copy of the untouched prefix
            for r in range(repeats):
                if r:
                    sp.wait_ge(s_all, 32 * r)
                sp.dma_start(out=out[:IDX, :], in_=cache[:IDX, :]).then_inc(
                    s_all, 16
                )
            sp.wait_ge(s_all, 32 * repeats - 16)

        @block.scalar
        def _(act):
            # small path on the second HWDGE ring, overlaps the SP copy
            for r in range(repeats):
                if r:
                    act.wait_ge(s_all, 32 * r)
                act.dma_start(out=a[:], in_=cache[IDX:TO, :]).then_inc(
                    s_load, 16
                )
                act.dma_start(out=b[:], in_=x[:, :]).then_inc(s_load, 16)
                act.wait_ge(s_add, r + 1)
                act.dma_start(out=out[IDX:TO, :], in_=c[:]).then_inc(s_all, 16)
            act.wait_ge(s_all, 32 * repeats)

        @block.vector
        def _(v):
            for r in range(repeats):
                v.wait_ge(s_load, 32 * (r + 1))
                v.tensor_add(c[:], a[:], b[:]).then_inc(s_add, 1)

    return nc


def kernel(cache, cache_mask, x, mask, index, reset_index, **_unused):
    global _NC
    assert int(index) == IDX and int(reset_index) == 0
    cache = np.asarray(cache, dtype=np.float32)
    x = np.asarray(x, dtype=np.float32)
    # Batch-shard: core i owns batch i. Only rows < TO are ever read.
    cache_s = np.ascontiguousarray(cache[:, :TO]).reshape(B, TO, F)
    x_s = np.ascontiguousarray(x).reshape(B, L, F)
    if _NC is None:
        _NC = _build()
    in_maps = [{"cache": cache_s[i], "x": x_s[i]} for i in range(N_CORES)]
    res = run_bass_kernel_spmd(_NC, in_maps, core_ids=list(range(N_CORES)))
    out = np.stack([res.results[i]["out"] for i in range(N_CORES)])
    return out.reshape(B, TO, H, D)
